# revision 56
# baseline (speedup 1.0000x reference)
"""ObjCondensationLoss Trainium2 kernel (8 NeuronCores, data-parallel over hits).

Reference semantics (N=100000 hits, K=256 clusters, D=3):
  L_beta = sum(1-beta_ak)/K + (S_b/N_b)*sum(beta[bg])
  q_i    = atanh(beta_i)^2 + q_min
  q_ak   = max_i q_i*M_ik ; x_a = x[argmax] (per cluster)
  L_v    = (1/N) sum_i q_i * sum_k (M*d2 + (1-M)*relu(1-d2)) * q_ak

Key identities used:
  - q is monotonic in beta  => one segment-argmax of beta gives beta_ak, q_ak, x_a.
  - q_i*q_k*relu(1-d2) = relu(q_i*q_k*(1-d2)) = relu(h_i . w_k)  with
      h_i = [q x0, q x1, q x2, q|x|^2, q],  w_k = [2q_k xa, -q_k, q_k(1-|xa|^2)]
    so the (N,K) potential matrix is ONE 5-contraction matmul + relu (bf16).
  - member (attractive) correction:
      Lv*N = sum_ik relu(v) + T1 - sum_mem v - sum_mem relu(v)
      T1         = sum_k q_ak * segsum_k(q_i)          (from HS matmul col 4)
      sum_mem v  = sum_kc w_kc * HS_kc,  HS = segsum of h vectors (one-hot matmul)
      sum_mem relu(v) = per-tile masked extract of relu(v), via the STORED
      phase-A bf16 masks and tensor_tensor_reduce.

Perf structure (v3):
  - phase A: bf16 one-hot mask (stored, reused in phase 2) + fused stt
    (mult beta, max) accumulate on DVE; bf16 segment-sum matmul on PE.
  - phase 2: bf16 5-contraction matmuls; relu split between ACT (accum tax
    187ns) and DVE-from-PSUM (free accumulator); extracts via stored masks.
"""

import numpy as np

N = 100000
K = 256
NC = 8
NLOC = N // NC          # 12500
P = 128
J = 98                  # P*J = 12544 padded local hits
NPAD = P * J
Q_MIN = 0.5
S_B = 1.0
BIG = np.float32(1 << 27)

_CACHE = {}

import os as _os
POOL_MAX = int(_os.environ.get("POOL_MAX", "0"))   # every POOL_MAXth j maxed on GPSIMD (0=off; TT illegal on Pool in this ISA)
BQ = int(_os.environ.get("BQ", "1"))               # j's per PSUM relu batch
ACT_MOD = int(_os.environ.get("ACT_MOD", "9"))     # of 5 relu batches: this many on ACT
ACT_CUT = int(_os.environ.get("ACT_CUT", "8"))     # batches b%ACT_MOD < ACT_CUT -> ACT
EXTRACT = _os.environ.get("EXTRACT", "stt")        # ttr | stt


def _build_nc():
    import concourse.bass as bass
    import concourse.bacc as bacc
    import concourse.mybir as mybir
    import concourse.tile as tile
    from concourse.masks import make_identity

    f32 = mybir.dt.float32
    i32 = mybir.dt.int32
    u32 = mybir.dt.uint32
    Alu = mybir.AluOpType
    Act = mybir.ActivationFunctionType
    Ax = mybir.AxisListType

    nc = bacc.Bacc()

    xs = nc.dram_tensor("xs", [P, 3 * J], f32, kind="ExternalInput")
    bs = nc.dram_tensor("bs", [P, J], f32, kind="ExternalInput")
    ys = nc.dram_tensor("ys", [P, J], i32, kind="ExternalInput")
    xf = nc.dram_tensor("xf", [N, 3], f32, kind="ExternalInput")
    out_dr = nc.dram_tensor("out", [1], f32, kind="ExternalOutput")

    ag_in = nc.dram_tensor("ag_in", [2, K], f32)
    ag_out = nc.dram_tensor("ag_out", [2 * NC, K], f32, addr_space="Shared")
    ar_in = nc.dram_tensor("ar_in", [1, 4], f32)
    ar_out = nc.dram_tensor("ar_out", [1, 4], f32, addr_space="Shared")
    dm_in = nc.dram_tensor("dm_in", [1, 4], f32)
    dm_out = nc.dram_tensor("dm_out", [1, 4], f32, addr_space="Shared")
    RG = [list(range(NC))]

    from contextlib import ExitStack
    with tile.TileContext(nc) as tc, ExitStack() as es:
        cp = es.enter_context(tc.tile_pool(name="cp", bufs=1))   # persistent sbuf
        mk = es.enter_context(tc.tile_pool(name="mk", bufs=8))   # scratch masks
        tr = es.enter_context(tc.tile_pool(name="tr", bufs=3))   # trash outputs
        pv = es.enter_context(tc.tile_pool(name="pv", bufs=2, space="PSUM"))
        pt = es.enter_context(tc.tile_pool(name="pt", bufs=2, space="PSUM"))
        ph = es.enter_context(tc.tile_pool(name="ph", bufs=1, space="PSUM"))
        _body(nc, tc, locals(), mybir, bass, make_identity,
              f32, i32, u32, Alu, Act, Ax,
              xs, bs, ys, xf, out_dr, ag_in, ag_out, ar_in, ar_out,
              dm_in, dm_out, RG,
              cp, mk, tr, pv, pt, ph)
    if not nc.is_finalized():
        nc.finalize()
    return nc


def _body(nc, tc, _loc, mybir, bass, make_identity,
          f32, i32, u32, Alu, Act, Ax,
          xs, bs, ys, xf, out_dr, ag_in, ag_out, ar_in, ar_out,
          dm_in, dm_out, RG,
          cp, mk, tr, pv, pt, ph):
    V = nc.vector
    S = nc.scalar
    G = nc.gpsimd
    T = nc.tensor
    KT = K // P  # 2 cluster tiles
    bf16 = mybir.dt.bfloat16

    # ---------------- constants & input loads ----------------
    identg = cp.tile([P, P], f32)
    make_identity(nc, identg[:])
    ident = cp.tile([P, P], f32)          # DVE-owned copy: single-sem deps
    V.tensor_copy(ident[:], identg[:])
    ident_bf = cp.tile([P, P], bf16)      # for bf16 transposes
    V.tensor_copy(ident_bf[:], identg[:])
    ones_col = cp.tile([P, 1], f32)
    V.memset(ones_col[:], 1.0)
    sgn = cp.tile([1, 11], f32)   # lv signs: [r,-mrel,T1,T1,-mv,-mv,0,0,0,0,-mcol]
    V.memset(sgn[:], 0.0)
    V.memset(sgn[:, 0:1], 1.0)
    V.memset(sgn[:, 1:2], -1.0)
    V.memset(sgn[:, 2:4], 1.0)
    V.memset(sgn[:, 4:6], -1.0)
    V.memset(sgn[:, 10:11], -1.0)

    iotaC_i = cp.tile([P, K], i32)       # 0..255 (mask cols = y)
    G.iota(iotaC_i[:], pattern=[[1, K]], base=0, channel_multiplier=0)
    iotaC_bf = cp.tile([P, K], bf16)
    V.tensor_copy(iotaC_bf[:], iotaC_i[:])

    jw_i = cp.tile([P, J], i32)          # 100 - j
    G.iota(jw_i[:], pattern=[[-1, J]], base=100, channel_multiplier=0)
    jw = cp.tile([P, J], f32)
    V.tensor_copy(jw[:], jw_i[:])

    x_sb = cp.tile([P, 3 * J], f32)      # (p, j*3+d) interleaved
    nc.sync.dma_start(out=x_sb[:], in_=xs[:])
    beta_sb = cp.tile([P, J], f32)
    nc.sync.dma_start(out=beta_sb[:], in_=bs[:])
    y_i = cp.tile([P, J], i32)
    nc.sync.dma_start(out=y_i[:], in_=ys[:])
    y_f = cp.tile([P, J], f32)
    V.tensor_copy(y_f[:], y_i[:])

    # ---------------- background stats (local) ----------------
    bgcol = cp.tile([P, 1], f32)
    tr98 = tr.tile([P, J], f32)
    V.scalar_tensor_tensor(out=tr98[:], in0=y_f[:], scalar=-1.0, in1=beta_sb[:],
                           op0=Alu.is_equal, op1=Alu.mult, accum_out=bgcol[:])
    nbcol = cp.tile([P, 1], f32)
    tr98b = tr.tile([P, J], f32)
    V.tensor_scalar(out=tr98b[:], in0=y_f[:], scalar1=-1.0, scalar2=None,
                    op0=Alu.is_equal, op1=Alu.add, accum_out=nbcol[:])

    # ---------------- q_i and staged hit vectors ----------------
    lnA = cp.tile([P, J], f32)
    S.activation(lnA[:], beta_sb[:], Act.Ln, bias=1.0, scale=1.0)
    lnB = cp.tile([P, J], f32)
    S.activation(lnB[:], beta_sb[:], Act.Ln, bias=1.0, scale=-1.0)
    ath = cp.tile([P, J], f32)
    V.tensor_tensor(out=ath[:], in0=lnA[:], in1=lnB[:], op=Alu.subtract)
    sq4 = cp.tile([P, J], f32)
    S.activation(sq4[:], ath[:], Act.Square, bias=0.0, scale=0.5)  # atanh^2
    q0 = cp.tile([P, J], f32)
    V.tensor_scalar(out=q0[:], in0=sq4[:], scalar1=Q_MIN, scalar2=None,
                    op0=Alu.add)
    validm = cp.tile([P, J], f32)        # y >= -1 (bg included, pads out)
    V.tensor_scalar(out=validm[:], in0=y_f[:], scalar1=-1.5, scalar2=None,
                    op0=Alu.is_gt)
    q_all = cp.tile([P, J], f32)
    V.tensor_tensor(out=q_all[:], in0=q0[:], in1=validm[:], op=Alu.mult)

    x0 = cp.tile([P, J], f32)
    x1 = cp.tile([P, J], f32)
    x2 = cp.tile([P, J], f32)
    V.tensor_copy(x0[:], x_sb[:, 0:3 * J:3])
    V.tensor_copy(x1[:], x_sb[:, 1:3 * J:3])
    V.tensor_copy(x2[:], x_sb[:, 2:3 * J:3])
    sqn = cp.tile([P, J], f32)
    tmpb = cp.tile([P, J], f32)
    V.tensor_tensor(out=sqn[:], in0=x0[:], in1=x0[:], op=Alu.mult)
    V.tensor_tensor(out=tmpb[:], in0=x1[:], in1=x1[:], op=Alu.mult)
    V.tensor_tensor(out=sqn[:], in0=sqn[:], in1=tmpb[:], op=Alu.add)
    V.tensor_tensor(out=tmpb[:], in0=x2[:], in1=x2[:], op=Alu.mult)
    V.tensor_tensor(out=sqn[:], in0=sqn[:], in1=tmpb[:], op=Alu.add)

    ST = 32  # stride per hit-vector: PE weight chunks must sit at base 0/32/64
    staging = cp.tile([P, ST * J], bf16)  # (p, j*32+c): [qx0,qx1,qx2,q|x|^2,q,...]
    V.memset(staging[:], 0.0)
    V.tensor_tensor(out=staging[:, 0:ST * J:ST], in0=x0[:], in1=q_all[:], op=Alu.mult)
    V.tensor_tensor(out=staging[:, 1:ST * J:ST], in0=x1[:], in1=q_all[:], op=Alu.mult)
    V.tensor_tensor(out=staging[:, 2:ST * J:ST], in0=x2[:], in1=q_all[:], op=Alu.mult)
    V.tensor_tensor(out=staging[:, 3:ST * J:ST], in0=sqn[:], in1=q_all[:], op=Alu.mult)
    V.tensor_copy(staging[:, 4:ST * J:ST], q_all[:])

    # dense (P, 5J) bf16 h vectors: lhsT slices for the HST seg-sum matmul
    hd = cp.tile([P, 5 * J], bf16)
    for c in range(5):
        V.tensor_copy(hd[:, c:5 * J:5], staging[:, c:ST * J:ST])

    # batch-transpose staged hit vectors: groups of 3 tiles (96 cols)
    groups = [(3 * g, 3) for g in range(32)] + [(96, 2)]
    hts = []
    for g0, gn in groups:
        pT = pt.tile([ST * gn, P], bf16, tag="pT")
        T.transpose(out=pT[:], in_=staging[:, ST * g0:ST * (g0 + gn)],
                    identity=ident_bf[:])
        hT = cp.tile([ST * gn, P], bf16, tag=f"hT{g0}")
        S.copy(hT[:], pT[:])
        hts.append(hT)

    # ---------------- pass A: segment max of beta ----------------
    # bf16 one-hot mask (stored for phase-2 extract reuse) + fused stt
    # (mult beta, max) accumulate on DVE; bf16 seg-sum matmul on PE.
    accA = cp.tile([P, K], f32)
    accD = cp.tile([P, K], f32)
    V.memset(accA[:], 0.0)
    V.memset(accD[:], 0.0)
    HST = ph.tile([5, K], f32, tag="HST")
    m2s = []
    for j in range(J):
        m2 = cp.tile([P, K], bf16, name=f"m2_{j}")
        V.tensor_scalar(out=m2[:], in0=iotaC_bf[:],
                        scalar1=y_f[:, j:j + 1], scalar2=None,
                        op0=Alu.is_equal)
        m2s.append(m2)
        # segment-sum of hit vectors (accumulated in PSUM across all j)
        T.matmul(out=HST[:], lhsT=hd[:, 5 * j:5 * j + 5], rhs=m2[:],
                 start=(j == 0), stop=(j == J - 1), skip_group_check=True)
        acc = accA if j % 2 == 0 else accD
        V.scalar_tensor_tensor(out=acc[:], in0=m2[:],
                               scalar=beta_sb[:, j:j + 1], in1=acc[:],
                               op0=Alu.mult, op1=Alu.max)
    V.tensor_tensor(out=accA[:], in0=accA[:], in1=accD[:], op=Alu.max)

    # transpose cluster columns 1..256 -> accT (cluster on partition)
    beta_loc, pstar = [], []
    for kt in range(KT):
        pT2 = pt.tile([P, P], f32, tag="pT")
        T.transpose(out=pT2[:], in_=accA[:, kt * P:(kt + 1) * P],
                    identity=ident[:])
        accTs = cp.tile([P, P], f32, tag=f"accTs{kt}")
        S.copy(accTs[:], pT2[:])
        top8 = cp.tile([P, 8], f32, tag=f"top8{kt}")
        V.max(top8[:], accTs[:])
        idx8 = cp.tile([P, 8], u32, tag=f"idx8{kt}")
        V.max_index(idx8[:], top8[:], accTs[:])
        bl = cp.tile([P, 1], f32, tag=f"bl{kt}")
        V.tensor_copy(bl[:], top8[:, 0:1])
        ps = cp.tile([P, 1], i32, tag=f"ps{kt}")
        V.tensor_copy(ps[:], idx8[:, 0:1])
        beta_loc.append(bl)
        pstar.append(ps)

    # local argmax recovery: gather the winner's beta row, find j*.
    # (No y-row check: fp32 betas are unique within a row w.p. 1; empty
    # clusters resolve to garbage gidx that the global select masks out.)
    gidx_loc = []
    for kt in range(KT):
        CR = cp.tile([P, J], f32, tag=f"CR{kt}")
        G.indirect_dma_start(
            out=CR[:], out_offset=None, in_=bs[:],
            in_offset=bass.IndirectOffsetOnAxis(ap=pstar[kt][:, 0:1], axis=0))
        w = cp.tile([P, J], f32, tag=f"w{kt}")
        V.tensor_scalar(out=w[:], in0=CR[:], scalar1=beta_loc[kt][:, 0:1],
                        scalar2=None, op0=Alu.is_equal)
        V.tensor_tensor(out=w[:], in0=w[:], in1=jw[:], op=Alu.mult)
        vmax = cp.tile([P, 1], f32, tag=f"vm{kt}")
        V.reduce_max(out=vmax[:], in_=w[:], axis=Ax.X)
        # jstar = 100 - v ; invalid (v==0) -> jstar=100 (harmless, masked below)
        jst = cp.tile([P, 1], f32, tag=f"jst{kt}")
        V.tensor_scalar(out=jst[:], in0=vmax[:], scalar1=-1.0, scalar2=100.0,
                        op0=Alu.mult, op1=Alu.add)
        valid = cp.tile([P, 1], f32, tag=f"vd{kt}")
        V.tensor_scalar(out=valid[:], in0=vmax[:], scalar1=0.5, scalar2=None,
                        op0=Alu.is_gt)
        psf = cp.tile([P, 1], f32, tag=f"psf{kt}")
        V.tensor_copy(psf[:], pstar[kt][:])
        gl = cp.tile([P, 1], f32, tag=f"gl{kt}")
        V.scalar_tensor_tensor(out=gl[:], in0=psf[:], scalar=float(J), in1=jst[:],
                               op0=Alu.mult, op1=Alu.add)
        # invalid -> BIG
        ivb = cp.tile([P, 1], f32, tag=f"ivb{kt}")
        V.tensor_scalar(out=ivb[:], in0=valid[:], scalar1=-float(BIG),
                        scalar2=float(BIG), op0=Alu.mult, op1=Alu.add)
        V.tensor_tensor(out=gl[:], in0=gl[:], in1=valid[:], op=Alu.mult)
        V.tensor_tensor(out=gl[:], in0=gl[:], in1=ivb[:], op=Alu.add)
        gidx_loc.append(gl)

    # ship local (beta, gidx) per cluster to all cores (single DMA/queue so the
    # collective trigger carries only one DMA wait)
    sb4 = cp.tile([P, 4], f32)
    V.tensor_copy(sb4[:, 0:1], beta_loc[0][:])
    V.tensor_copy(sb4[:, 1:2], beta_loc[1][:])
    V.tensor_copy(sb4[:, 2:3], gidx_loc[0][:])
    V.tensor_copy(sb4[:, 3:4], gidx_loc[1][:])
    nc.sync.dma_start(out=ag_in[:].rearrange("a (c p) -> p (a c)", p=P),
                      in_=sb4[:])
    G.collective_compute("AllGather", mybir.AluOpType.bypass,
                         replica_groups=RG, ins=[ag_in[:]], outs=[ag_out[:]])

    bsrc = cp.tile([NC, K], f32)
    nc.sync.dma_start(out=bsrc[:], in_=ag_out[0:2 * NC:2, :])
    gsrc = cp.tile([NC, K], f32)
    nc.sync.dma_start(out=gsrc[:], in_=ag_out[1:2 * NC:2, :])
    roff_i = cp.tile([NC, 1], i32)
    G.iota(roff_i[:], pattern=[[1, 1]], base=0, channel_multiplier=NLOC)
    roff = cp.tile([NC, 1], f32)
    V.tensor_copy(roff[:], roff_i[:])
    V.tensor_scalar(out=gsrc[:], in0=gsrc[:], scalar1=roff[:, 0:1],
                    scalar2=None, op0=Alu.add)

    # global select per cluster tile: max beta, tie -> lowest rank
    beta_g, gidx_g, one_m, q_ak, W = [], [], [], [], []
    for kt in range(KT):
        pT2 = pt.tile([P, NC], f32, tag="pT")
        T.transpose(out=pT2[:], in_=bsrc[:, kt * P:(kt + 1) * P],
                    identity=ident[0:NC, 0:NC])
        bt = cp.tile([P, NC], f32, tag=f"bt{kt}")
        S.copy(bt[:], pT2[:])
        pT3 = pt.tile([P, NC], f32, tag="pT")
        T.transpose(out=pT3[:], in_=gsrc[:, kt * P:(kt + 1) * P],
                    identity=ident[0:NC, 0:NC])
        gt = cp.tile([P, NC], f32, tag=f"gt{kt}")
        S.copy(gt[:], pT3[:])
        w_cur, g_cur, width = bt, gt, NC
        while width > 1:
            h = width // 2
            ge = cp.tile([P, h], f32, tag=f"ge{kt}_{h}")
            V.tensor_tensor(out=ge[:], in0=w_cur[:, 0:h], in1=w_cur[:, h:width],
                            op=Alu.is_ge)
            bnew = cp.tile([P, h], f32, tag=f"bn{kt}_{h}")
            V.tensor_tensor(out=bnew[:], in0=w_cur[:, 0:h], in1=w_cur[:, h:width],
                            op=Alu.max)
            gd = cp.tile([P, h], f32, tag=f"gd{kt}_{h}")
            V.tensor_tensor(out=gd[:], in0=g_cur[:, 0:h], in1=g_cur[:, h:width],
                            op=Alu.subtract)
            V.tensor_tensor(out=gd[:], in0=ge[:], in1=gd[:], op=Alu.mult)
            gnew = cp.tile([P, h], f32, tag=f"gn{kt}_{h}")
            V.tensor_tensor(out=gnew[:], in0=gd[:], in1=g_cur[:, h:width],
                            op=Alu.add)
            w_cur, g_cur, width = bnew, gnew, h
        bg_t = cp.tile([P, 1], f32, tag=f"bg{kt}")
        V.tensor_copy(bg_t[:], w_cur[:, 0:1])
        beta_g.append(bg_t)
        emt = cp.tile([P, 1], f32, tag=f"em{kt}")
        V.tensor_scalar(out=emt[:], in0=bg_t[:], scalar1=0.0, scalar2=None,
                        op0=Alu.is_equal)
        om = cp.tile([P, 1], f32, tag=f"om{kt}")
        V.tensor_scalar(out=om[:], in0=emt[:], scalar1=-1.0, scalar2=1.0,
                        op0=Alu.mult, op1=Alu.add)
        one_m.append(om)
        gg = cp.tile([P, 1], f32, tag=f"gg{kt}")
        V.tensor_tensor(out=gg[:], in0=g_cur[:, 0:1], in1=om[:], op=Alu.mult)
        gidx_g.append(gg)

        # q_ak = (1-empty) * (atanh(beta_g)^2 + qmin)
        la = cp.tile([P, 1], f32, tag=f"la{kt}")
        S.activation(la[:], bg_t[:], Act.Ln, bias=1.0, scale=1.0)
        lb = cp.tile([P, 1], f32, tag=f"lb{kt}")
        S.activation(lb[:], bg_t[:], Act.Ln, bias=1.0, scale=-1.0)
        at = cp.tile([P, 1], f32, tag=f"at{kt}")
        V.tensor_tensor(out=at[:], in0=la[:], in1=lb[:], op=Alu.subtract)
        s4 = cp.tile([P, 1], f32, tag=f"s4{kt}")
        S.activation(s4[:], at[:], Act.Square, bias=0.0, scale=0.5)
        qa0 = cp.tile([P, 1], f32, tag=f"qa0{kt}")
        V.tensor_scalar(out=qa0[:], in0=s4[:], scalar1=Q_MIN, scalar2=None,
                        op0=Alu.add)
        qa = cp.tile([P, 1], f32, tag=f"qa{kt}")
        V.tensor_tensor(out=qa[:], in0=qa0[:], in1=om[:], op=Alu.mult)
        q_ak.append(qa)

        # gather x_a rows
        gi = cp.tile([P, 1], i32, tag=f"gi{kt}")
        V.tensor_copy(gi[:], gg[:])
        xa = cp.tile([P, 3], f32, tag=f"xa{kt}")
        G.indirect_dma_start(
            out=xa[:], out_offset=None, in_=xf[:],
            in_offset=bass.IndirectOffsetOnAxis(ap=gi[:, 0:1], axis=0))
        # cluster weight vectors
        xx = cp.tile([P, 3], f32, tag=f"xx{kt}")
        V.tensor_tensor(out=xx[:], in0=xa[:], in1=xa[:], op=Alu.mult)
        sn = cp.tile([P, 1], f32, tag=f"sn{kt}")
        V.reduce_sum(out=sn[:], in_=xx[:], axis=Ax.X)
        q2 = cp.tile([P, 1], f32, tag=f"q2{kt}")
        V.tensor_scalar(out=q2[:], in0=qa[:], scalar1=2.0, scalar2=None,
                        op0=Alu.mult)
        Wk = cp.tile([P, 5], f32, tag=f"W{kt}")
        V.tensor_scalar(out=Wk[:, 0:3], in0=xa[:], scalar1=q2[:, 0:1],
                        scalar2=None, op0=Alu.mult)
        V.tensor_scalar(out=Wk[:, 3:4], in0=qa[:], scalar1=-1.0, scalar2=None,
                        op0=Alu.mult)
        t1m = cp.tile([P, 1], f32, tag=f"t1m{kt}")
        V.tensor_scalar(out=t1m[:], in0=sn[:], scalar1=-1.0, scalar2=1.0,
                        op0=Alu.mult, op1=Alu.add)
        V.tensor_tensor(out=Wk[:, 4:5], in0=t1m[:], in1=qa[:], op=Alu.mult)
        W.append(Wk)

    # Wall (5, 256): transposed cluster weights, both tiles side by side
    wallp = ph.tile([5, K], bf16, tag="wallp")
    wallt = cp.tile([P, 5 * KT], bf16)
    for kt in range(KT):
        V.tensor_copy(wallt[:, 5 * kt:5 * kt + 5], W[kt][:])
    for kt in range(KT):
        T.transpose(out=wallp[:, kt * P:(kt + 1) * P],
                    in_=wallt[:, 5 * kt:5 * kt + 5],
                    identity=ident_bf[:])
    wall3 = cp.tile([69, K], bf16)
    for b in (0, 32, 64):
        S.copy(wall3[b:b + 5, :], wallp[:])

    # SUM assembly of everything already available (pre-phase-2, shortens tail)
    SUM = cp.tile([P, 11], f32)
    G.memset(SUM[:], 0.0)
    hs_sb = cp.tile([5, K], f32)
    S.copy(hs_sb[:], HST[:])
    for kt in range(KT):
        pT4 = pt.tile([P, 5], f32, tag="pT")
        T.transpose(out=pT4[:], in_=hs_sb[:, kt * P:(kt + 1) * P],
                    identity=ident[0:5, 0:5])
        hst_t = cp.tile([P, 5], f32, tag=f"hstt{kt}")
        S.copy(hst_t[:], pT4[:])
        # T1 col: q_ak * segsum_q
        V.tensor_tensor(out=SUM[:, 2 + kt:3 + kt], in0=q_ak[kt][:],
                        in1=hst_t[:, 4:5], op=Alu.mult)
        # sum_mem v col: dot(W_k, HS_k)
        wdot = cp.tile([P, 5], f32, tag=f"wdot{kt}")
        V.tensor_tensor(out=wdot[:], in0=W[kt][:], in1=hst_t[:], op=Alu.mult)
        V.reduce_sum(out=SUM[:, 4 + kt:5 + kt], in_=wdot[:], axis=Ax.X)
        # sbet col: (1 - beta_g)
        V.tensor_scalar(out=SUM[:, 8 + kt:9 + kt], in0=beta_g[kt][:],
                        scalar1=-1.0, scalar2=1.0, op0=Alu.mult, op1=Alu.add)
    V.tensor_copy(SUM[:, 6:7], bgcol[:])
    V.tensor_copy(SUM[:, 7:8], nbcol[:])

    # block-diagonal pair weights: pair (j0, j1) in one hts tile ->
    # ONE 512-col matmul with lhsT = hts[g][0:37] (j0 h at rows 0-4,
    # j1 h at rows 32-36, zeros between) and rhs = wall2 (37, 512).
    wall2 = cp.tile([37, 2 * K], bf16)
    V.memset(wall2[:], 0.0)
    S.copy(wall2[0:5, 0:K], wallp[:])
    S.copy(wall2[32:37, K:2 * K], wallp[:])

    # ---------------- phase 2: potential matmul ----------------
    # per unit (pair of j's or a single j): matmul(s) -> relu (ACT/DVE
    # split, rcol accum); member-relu total via Frobenius: t_j = m2_j *
    # ta_j (bf16 TT at 2x), column-summed by an accumulating ones-matmul.
    units = []
    for g in range(33):
        units.append((g, (3 * g, 3 * g + 1)))
        if 3 * g + 2 < J:
            units.append((g, (3 * g + 2,)))
    rcol = cp.tile([P, len(units)], f32)
    mcol = cp.tile([P, J], f32)
    G.memset(mcol[:], 0.0)
    ones_bf = cp.tile([P, 1], bf16)
    V.memset(ones_bf[:], 1.0)
    mrelP = ph.tile([1, 2 * K], f32, tag="mrelP")
    td4_cur = [None]
    # mrel ones-matmuls are emitted LAGGED so the in-order PE queue never
    # stalls waiting for the DVE mask*relu products.
    mrel_q = []
    mrel_n = [0]
    pend_td = [None]

    def flush_mrel(final=False):
        while mrel_q and (final or len(mrel_q) > 3):
            td4t, w4 = mrel_q.pop(0)
            T.matmul(out=mrelP[:, 0:w4 * K], lhsT=ones_bf[:],
                     rhs=td4t[:, 0:w4 * K],
                     start=(mrel_n[0] == 0), stop=(final and not mrel_q),
                     skip_group_check=True)
            mrel_n[0] += 1

    for u, (g, js) in enumerate(units):
        flush_mrel()
        W2 = len(js) * K
        pvt4 = pv.tile([P, W2], f32, tag="pvt4", name=f"pvt4_{u}")
        if len(js) == 2:
            T.matmul(out=pvt4[:], lhsT=hts[g][0:37, :], rhs=wall2[:],
                     start=True, stop=True, skip_group_check=True)
        else:
            jj = js[0] % 3
            T.matmul(out=pvt4[:], lhsT=hts[g][32 * jj:32 * jj + 5, :],
                     rhs=wall3[32 * jj:32 * jj + 5, :],
                     start=True, stop=True, skip_group_check=True)
        ta = tr.tile([P, W2], bf16, tag="ta", name=f"ta_{u}")
        if u % ACT_MOD < ACT_CUT:
            S.activation(ta[:], pvt4[:], Act.Relu, accum_out=rcol[:, u:u + 1])
        else:
            V.tensor_scalar(out=ta[:], in0=pvt4[:], scalar1=0.0, scalar2=None,
                            op0=Alu.max, op1=Alu.add,
                            accum_out=rcol[:, u:u + 1])
        for q, j in enumerate(js):
            if (j // 2) % 3 == 0:
                # Frobenius path: mask*relu product, column-summed by PE
                if j % 2 == 0:
                    td4_cur[0] = tr.tile([P, 2 * K], bf16, tag="td4",
                                         name=f"td4_{j}", bufs=6)
                td4 = td4_cur[0]
                V.tensor_tensor(out=td4[:, (j % 2) * K:(j % 2 + 1) * K],
                                in0=m2s[j][:],
                                in1=ta[:, q * K:(q + 1) * K], op=Alu.mult)
                if j % 2 == 1 or j == J - 1:
                    mrel_q.append((td4, (j % 2) + 1))
            else:
                # stt extract path: accumulate member relu into mcol
                td = tr.tile([P, K], bf16, tag="td", name=f"td_{j}")
                V.scalar_tensor_tensor(out=td[:], in0=iotaC_bf[:],
                                       scalar=y_f[:, j:j + 1],
                                       in1=ta[:, q * K:(q + 1) * K],
                                       op0=Alu.is_equal, op1=Alu.mult,
                                       accum_out=mcol[:, j:j + 1])
    flush_mrel(final=True)

    # ---------------- reductions & loss ----------------
    V.reduce_sum(out=SUM[:, 0:1], in_=rcol[:], axis=Ax.X)
    mrel_sb = cp.tile([1, 2 * K], f32)
    S.copy(mrel_sb[:], mrelP[:])
    V.reduce_sum(out=SUM[0:1, 1:2], in_=mrel_sb[:], axis=Ax.X)
    V.reduce_sum(out=SUM[:, 10:11], in_=mcol[:], axis=Ax.X)

    SUMa = cp.tile([P, 11], f32)
    S.copy(SUMa[:], SUM[:])                # ACT launder: sum-matmul waits 1 sem
    sump = ph.tile([1, 11], f32, tag="sump")
    T.matmul(out=sump[:], lhsT=ones_col[:], rhs=SUMa[:], start=True, stop=True)
    sums = cp.tile([1, 11], f32)
    S.copy(sums[:], sump[:])

    # lv_loc = r + T1(0) + T1(1) - mv0 - mv1 - mrel_frob - mrel_stt
    lvt = cp.tile([1, 11], f32)
    V.tensor_tensor(out=lvt[:], in0=sums[:], in1=sgn[:], op=Alu.mult)
    lv = cp.tile([1, 1], f32)
    V.reduce_sum(out=lv[:], in_=lvt[:], axis=Ax.X)

    arp = cp.tile([1, 4], f32)
    G.memset(arp[:], 0.0)
    V.tensor_copy(arp[:, 0:1], lv[:])
    V.tensor_copy(arp[:, 1:2], sums[0:1, 6:7])
    V.tensor_copy(arp[:, 2:3], sums[0:1, 7:8])
    nc.sync.dma_start(out=ar_in[:], in_=arp[:])
    G.collective_compute("AllReduce", mybir.AluOpType.add,
                         replica_groups=RG, ins=[ar_in[:]], outs=[ar_out[:]])
    ars = cp.tile([1, 4], f32)
    nc.sync.dma_start(out=ars[:], in_=ar_out[:])

    nbinv = cp.tile([1, 1], f32)
    V.reciprocal(out=nbinv[:], in_=ars[:, 2:3])
    tbg = cp.tile([1, 1], f32)
    V.tensor_tensor(out=tbg[:], in0=ars[:, 1:2], in1=nbinv[:], op=Alu.mult)
    V.tensor_scalar(out=tbg[:], in0=tbg[:], scalar1=float(S_B), scalar2=None,
                    op0=Alu.mult)
    sbet = cp.tile([1, 1], f32)
    V.tensor_tensor(out=sbet[:], in0=sums[0:1, 8:9], in1=sums[0:1, 9:10], op=Alu.add)
    loss = cp.tile([1, 1], f32)
    V.tensor_scalar(out=loss[:], in0=sbet[:], scalar1=1.0 / K, scalar2=None,
                    op0=Alu.mult)
    V.tensor_tensor(out=loss[:], in0=loss[:], in1=tbg[:], op=Alu.add)
    tlv = cp.tile([1, 1], f32)
    V.tensor_scalar(out=tlv[:], in0=ars[:, 0:1], scalar1=float(np.float32(1.0 / N)),
                    scalar2=None, op0=Alu.mult)
    V.tensor_tensor(out=loss[:], in0=loss[:], in1=tlv[:], op=Alu.add)
    nc.sync.dma_start(out=out_dr[None, :], in_=loss[:])


def _shard_inputs(x, beta, y):
    x = np.ascontiguousarray(np.asarray(x, dtype=np.float32))
    beta = np.ascontiguousarray(np.asarray(beta, dtype=np.float32))
    y = np.ascontiguousarray(np.asarray(y)).astype(np.int32)
    in_maps = []
    for r in range(NC):
        sl = slice(r * NLOC, (r + 1) * NLOC)
        xp = np.zeros((NPAD, 3), np.float32)
        bp = np.zeros((NPAD,), np.float32)
        yp = np.full((NPAD,), -2, np.int32)
        xp[:NLOC] = x[sl]
        bp[:NLOC] = beta[sl]
        yp[:NLOC] = y[sl]
        in_maps.append({
            "xs": xp.reshape(P, 3 * J),
            "bs": bp.reshape(P, J),
            "ys": yp.reshape(P, J),
            "xf": x,
        })
    return in_maps


def _install_ntff_hook_shim():
    """antenv.axon_hooks is absent in this image; recreate it via ctypes
    so run_bass_kernel_spmd(trace=True) can capture NTFF profiles."""
    import sys
    import types
    try:
        import antenv.axon_hooks  # noqa: F401
        return
    except ImportError:
        pass
    try:
        import antenv
        from trn_agent_boot.trn_boot import _ntff_profile_via_ctypes
        hook = _ntff_profile_via_ctypes("/opt/axon/libaxon_pjrt.so")
        mod = types.ModuleType("antenv.axon_hooks")
        mod._hook = hook
        mod.get_axon_ntff_profile_hook = lambda: mod._hook
        mod.set_axon_ntff_profile_hook = lambda h: setattr(mod, "_hook", h)
        sys.modules["antenv.axon_hooks"] = mod
        antenv.axon_hooks = mod
    except Exception as e:  # degrade to no tracing
        print(f"ntff hook shim failed: {e}")


def kernel(x, beta, y, K=256, S_b=1.0, q_min=0.5):
    import os
    assert int(K) == 256 and float(S_b) == 1.0 and float(q_min) == 0.5
    if int(os.environ.get("KERNEL_TRACE", "0")):
        _install_ntff_hook_shim()
    if "nc" not in _CACHE:
        _CACHE["nc"] = _build_nc()
    from concourse.bass_utils import run_bass_kernel_spmd
    in_maps = _shard_inputs(x, beta, y)
    trace = bool(int(os.environ.get("KERNEL_TRACE", "0")))
    res = run_bass_kernel_spmd(_CACHE["nc"], in_maps, core_ids=list(range(NC)),
                               trace=trace)
    _CACHE["last_results"] = res
    return np.float32(np.asarray(res.results[0]["out"]).reshape(-1)[0])


def run_sim(x, beta, y):
    """Multi-core simulator run (no hardware)."""
    import concourse.bass_interp as bass_interp
    if "nc" not in _CACHE:
        _CACHE["nc"] = _build_nc()
    nc = _CACHE["nc"]
    in_maps = _shard_inputs(x, beta, y)
    sim = bass_interp.MultiCoreSim(nc, NC)
    for r in range(NC):
        for k, v in in_maps[r].items():
            sim.cores[r].tensor(k)[:] = v
    sim.simulate()
    return np.float32(np.asarray(sim.cores[0].mem_tensor("out")).reshape(-1)[0])


if __name__ == "__main__":
    import sys
    sys.path.insert(0, "/root/problem")
    import jax
    import reference
    with jax.default_device(jax.devices("cpu")[0]):
        inputs = reference.setup_inputs()
        inputs = {k: (np.asarray(v) if hasattr(v, "shape") else v)
                  for k, v in inputs.items()}
        expected = float(reference.reference(**inputs))
    if "--sim" in sys.argv:
        got = float(run_sim(inputs["x"], inputs["beta"], inputs["y"]))
    else:
        got = float(kernel(**{k: (np.asarray(v) if hasattr(v, "shape") else v)
                              for k, v in inputs.items()}))
    rel = abs(got - expected) / max(abs(expected), 1e-30)
    print(f"expected={expected!r} got={got!r} rel={rel:.3e}")
